# revision 1
# baseline (speedup 1.0000x reference)
"""Trainium2 Bass kernel for CenterHead loss (data-parallel over batch, 8 cores).

Math notes
----------
reference loss = focal(sigmoid(preds[:,0]), target_hm) + 2 * L1(pred_reg, target_reg)

The target heatmap is 0 everywhere except a 3x3 patch per batch (center 1.0,
ring 0.8), and target_reg/mask are nonzero only at the center pixel. So:
  * neg-loss base: treat EVERY pixel of channel 0 as a t=0 negative:
      sum log(1-p) * p^2   over all pixels
    computed as -sum softplus(x) * p^2 with
      e = exp(-x); L = ln(1+e) (=softplus(-x)); p^2 = exp(-2L); softplus(x) = x+L
    (single ACT table: natural_log_exp_and_others; no table switching)
  * corrections for the <=9 patch pixels per batch:
      ring pixel (t=0.8, in range):  weight changes 1 -> 0.2^4
      center (t=1.0): remove its neg term, add pos term ln(p)*(1-p)^2
  * reg L1 needs preds[b,1:7,cy,cx] plus targets from gt_boxes
    (floor/ln/sin-cos-poly computed on device).

The host ships preds TRANSPOSED to (B, H, C, W) so that, per batch, image rows
start..start+2 (start = clip(cy-1, 0, H-3)) are one contiguous 3*C*W slab that
contains the channel-0 patch rows AND all six reg rows. One indirect DMA with
64 descriptors (one per batch) fetches everything data-dependent; channels 1..6
are never streamed, so the kernel reads ~1/7 of preds.

Slab slot k holds image row y_k = start+k; at the y-edges the slots shift, so
all patch/center masks are computed from y_k vs cy (slot-shift handling).

Per-core output "partials" [128, 8] f32 columns:
  0: per-partition sum of softplus(x)*p^2 (= -neg_base partial)
  1: per-batch neg-loss correction     2: per-batch pos term
  3: per-batch reg L1                  4: per-batch valid flag
Host sums across partitions+cores and applies the final divisions.
"""
from contextlib import ExitStack

import numpy as np

import concourse.bass as bass
import concourse.bacc as bacc
import concourse.tile as tile
import concourse.mybir as mybir

f32 = mybir.dt.float32
i32 = mybir.dt.int32
AF = mybir.ActivationFunctionType
OP = mybir.AluOpType
AX = mybir.AxisListType

B, C, H, W = 512, 7, 128, 128
NCORES = 8
BS = B // NCORES            # 64 batches per core
RPB = C * W                 # 896 elems per (b,y) row in transposed layout
ROWS = BS * H               # 8192 rows of the [BS*H, C*W] view
NT = 4                      # streaming tiles
TB = BS // NT               # 16 batches per tile
FD = TB * H * W // 128      # 2048 free elems per partition per tile

W4M1 = float((1.0 - 0.8) ** 4 - 1.0)   # ring weight delta: (1-t)^4 - 1

# sin/cos via polynomial in u=v^2, v = yaw - pi in [-pi,pi]:
#   sin(yaw) = -v*P(u), cos(yaw) = -Q(u)
def _trig_coefs():
    import numpy.polynomial.chebyshev as cheb
    vg = np.linspace(-np.pi, np.pi, 20001)
    sin_c = np.polynomial.Polynomial(cheb.cheb2poly(cheb.chebfit(vg**2, np.sinc(vg / np.pi), 6))).coef
    cos_c = np.polynomial.Polynomial(cheb.cheb2poly(cheb.chebfit(vg**2, np.cos(vg), 7))).coef
    return [float(c) for c in sin_c], [float(c) for c in cos_c]

SIN_C, COS_C = _trig_coefs()


def _body(ctx: ExitStack, tc, preds, gt, out):
    nc = tc.nc
    xp = ctx.enter_context(tc.tile_pool(name="xp", bufs=3))
    big = ctx.enter_context(tc.tile_pool(name="big", bufs=2))
    sm = ctx.enter_context(tc.tile_pool(name="sm", bufs=1))

    def _mk(pool):
        def f(shape, dtype, tag):
            return pool.tile(shape, dtype, tag=tag, name=tag)
        return f
    sm_tile, xp_tile, big_tile = _mk(sm), _mk(xp), _mk(big)

    partials = sm_tile([128, 8], f32, "partials")
    nc.vector.memset(partials[:], 0.0)

    # ---------------- big streaming pass over channel 0 ----------------
    # sum softplus(x)*p^2 = sum (x+L)*R accumulated on the PE as
    # diag(sum_chunks x_c.T @ R_c) + diag(sum_chunks L_c.T @ R_c)
    psum = ctx.enter_context(tc.tile_pool(name="psum", bufs=1, space="PSUM"))
    psA = psum.tile([128, 128], f32, tag="psA", name="psA")
    psB = psum.tile([128, 128], f32, tag="psB", name="psB")
    ident = sm_tile([128, 128], f32, "ident")
    from concourse.masks import make_identity
    make_identity(nc, ident[:])
    # preds is the (BS*H, C*W) view of (BS, H, C, W); ch0 = first W of each row
    hmv = preds.rearrange("(b y) cx -> b y cx", y=H)[:, :, 0:W]   # (BS,H,W)
    NCH = FD // 128
    for t in range(NT):
        x = xp_tile([128, FD], f32, "x")
        src = hmv[t * TB:(t + 1) * TB].rearrange("b y x -> y b x")
        nc.sync.dma_start(x[:].rearrange("p (b x) -> p b x", x=W), src)
        e = big_tile([128, FD], f32, "e")
        nc.scalar.activation(e[:], x[:], AF.Exp, scale=-1.0)
        L = big_tile([128, FD], f32, "L")
        nc.scalar.activation(L[:], e[:], AF.Ln, bias=1.0)
        R = big_tile([128, FD], f32, "R")
        nc.scalar.activation(R[:], L[:], AF.Exp, scale=-2.0)
        for cchunk in range(NCH):
            cs = slice(cchunk * 128, (cchunk + 1) * 128)
            first = (t == 0 and cchunk == 0)
            last = (t == NT - 1 and cchunk == NCH - 1)
            nc.tensor.matmul(psA[:], x[:, cs], R[:, cs], start=first, stop=last)
            nc.tensor.matmul(psB[:], L[:, cs], R[:, cs], start=first, stop=last)
    scrd = sm_tile([128, 128], f32, "scrd")
    nc.vector.scalar_tensor_tensor(
        out=scrd[:], in0=psA[:], scalar=1.0, in1=ident[:],
        op0=OP.mult, op1=OP.mult, accum_out=partials[:, 0:1])
    nc.vector.scalar_tensor_tensor(
        out=scrd[:], in0=psB[:], scalar=1.0, in1=ident[:],
        op0=OP.mult, op1=OP.mult, accum_out=partials[:, 5:6])

    # ---------------- per-batch values from gt_boxes ----------------
    gtt = sm_tile([BS, 6], f32, "gtt")
    nc.sync.dma_start(gtt[:], gt[:])
    cxf, cyf = gtt[:, 1:2], gtt[:, 2:3]

    # floor of (cx, cy) together: round via f32->i32 copy, fix up if rf > src
    fl_i = sm_tile([BS, 2], i32, "fl_i")
    nc.vector.tensor_copy(fl_i[:], gtt[:, 1:3])
    fl_f = sm_tile([BS, 2], f32, "fl_f")
    nc.vector.tensor_copy(fl_f[:], fl_i[:])
    fl_fx = sm_tile([BS, 2], f32, "fl_fx")
    nc.vector.tensor_tensor(out=fl_fx[:], in0=fl_f[:], in1=gtt[:, 1:3], op=OP.is_gt)
    nc.vector.tensor_tensor(out=fl_f[:], in0=fl_f[:], in1=fl_fx[:], op=OP.subtract)
    nc.vector.tensor_copy(fl_i[:], fl_f[:])
    cx_f, cy_f = fl_f[:, 0:1], fl_f[:, 1:2]
    cy_i = fl_i[:, 1:2]

    # valid = 0 <= cx < W and 0 <= cy < H (W == H == 128 so one bound tile)
    vboth = sm_tile([BS, 2], f32, "vboth")
    vtmp = sm_tile([BS, 2], f32, "vtmp")
    nc.vector.tensor_scalar(out=vboth[:], in0=gtt[:, 1:3], scalar1=0.0, scalar2=None, op0=OP.is_ge)
    nc.vector.tensor_scalar(out=vtmp[:], in0=gtt[:, 1:3], scalar1=float(W), scalar2=None, op0=OP.is_lt)
    nc.vector.tensor_tensor(out=vboth[:], in0=vboth[:], in1=vtmp[:], op=OP.mult)
    vf = sm_tile([BS, 1], f32, "vf")
    nc.vector.tensor_tensor(out=vf[:], in0=vboth[:, 0:1], in1=vboth[:, 1:2], op=OP.mult)

    # slab start row: start = clip(cy-1, 0, H-3); gather row index = b*H + start
    st_i = sm_tile([BS, 1], i32, "st_i")
    nc.vector.tensor_scalar(out=st_i[:], in0=cy_i, scalar1=-1, scalar2=0,
                            op0=OP.add, op1=OP.max)
    nc.vector.tensor_scalar(out=st_i[:], in0=st_i[:], scalar1=H - 3, scalar2=None, op0=OP.min)
    st_f = sm_tile([BS, 1], f32, "st_f")
    nc.vector.tensor_copy(st_f[:], st_i[:])
    biota = sm_tile([BS, 1], i32, "biota")
    nc.gpsimd.iota(biota[:], pattern=[[0, 1]], base=0, channel_multiplier=H)
    gidx = sm_tile([BS, 1], i32, "gidx")
    nc.vector.tensor_tensor(out=gidx[:], in0=st_i[:], in1=biota[:], op=OP.add)

    # one slab gather: 3 view-rows (3*C*W elems) per batch
    slab = sm_tile([BS, 3 * RPB], f32, "slab")
    nc.gpsimd.indirect_dma_start(
        out=slab[:], out_offset=None, in_=preds[:],
        in_offset=bass.IndirectOffsetOnAxis(ap=gidx[:, 0:1], axis=0))

    def slab_ch(k, c):  # (BS, W) AP of slot k, channel c
        return slab[:, k * RPB + c * W: k * RPB + (c + 1) * W]

    # slot masks vs cy: mk = [y_k == cy], rowmask_k = [|y_k - cy| <= 1]
    mk, rowm = [], []
    for k in range(3):
        m = sm_tile([BS, 1], f32, f"mk{k}")
        nc.vector.tensor_scalar(out=m[:], in0=st_f[:], scalar1=float(k), scalar2=cy_f,
                                op0=OP.add, op1=OP.is_equal)
        mk.append(m)
        r1 = sm_tile([BS, 1], f32, f"rma{k}")
        nc.vector.tensor_scalar(out=r1[:], in0=st_f[:], scalar1=float(k + 1), scalar2=cy_f,
                                op0=OP.add, op1=OP.is_ge)
        r2 = sm_tile([BS, 1], f32, f"rmb{k}")
        nc.vector.tensor_scalar(out=r2[:], in0=st_f[:], scalar1=float(k - 1), scalar2=cy_f,
                                op0=OP.add, op1=OP.is_le)
        nc.vector.tensor_tensor(out=r1[:], in0=r1[:], in1=r2[:], op=OP.mult)
        rowm.append(r1)

    # col-ok masks and x-onehots per dx (onehot [x - dx == cx] needs no clip)
    iota_x = sm_tile([BS, W], i32, "iota_x")
    nc.gpsimd.iota(iota_x[:], pattern=[[1, W]], base=0, channel_multiplier=0)
    iota_xf = sm_tile([BS, W], f32, "iota_xf")
    nc.vector.tensor_copy(iota_xf[:], iota_x[:])
    oh, colok = {}, {}
    for dx in (-1, 0, 1):
        o = sm_tile([BS, W], f32, f"oh{dx}")
        nc.vector.tensor_scalar(out=o[:], in0=iota_xf[:], scalar1=float(-dx), scalar2=cx_f,
                                op0=OP.add, op1=OP.is_equal)
        oh[dx] = o
        ck1 = sm_tile([BS, 1], f32, f"cka{dx}")
        nc.vector.tensor_scalar(out=ck1[:], in0=cx_f, scalar1=float(dx), scalar2=0.0,
                                op0=OP.add, op1=OP.is_ge)
        ck2 = sm_tile([BS, 1], f32, f"ckb{dx}")
        nc.vector.tensor_scalar(out=ck2[:], in0=cx_f, scalar1=float(dx), scalar2=float(W - 1),
                                op0=OP.add, op1=OP.is_le)
        nc.vector.tensor_tensor(out=ck1[:], in0=ck1[:], in1=ck2[:], op=OP.mult)
        colok[dx] = ck1

    # extract the 9 patch logits X[:, j], j = k*3 + (dx+1)
    X = sm_tile([BS, 9], f32, "X")
    scr = sm_tile([BS, W], f32, "scr")
    for k in range(3):
        for dx in (-1, 0, 1):
            j = k * 3 + (dx + 1)
            nc.vector.scalar_tensor_tensor(
                out=scr[:], in0=slab_ch(k, 0), scalar=1.0, in1=oh[dx][:],
                op0=OP.mult, op1=OP.mult, accum_out=X[:, j:j + 1])

    # weights: W9 = w4m1*basemask - (w4m1+1)*centermask
    #   basemask_j = rowmask_k * colok_dx * valid; centermask_j = mk * [dx==0] * valid
    W9 = sm_tile([BS, 9], f32, "W9")
    C9 = sm_tile([BS, 9], f32, "C9")
    rvk = sm_tile([BS, 3], f32, "rvk")
    mvk = sm_tile([BS, 3], f32, "mvk")
    for k in range(3):
        nc.vector.tensor_tensor(out=rvk[:, k:k + 1], in0=rowm[k][:], in1=vf[:], op=OP.mult)
        nc.vector.tensor_tensor(out=mvk[:, k:k + 1], in0=mk[k][:], in1=vf[:], op=OP.mult)
    nc.vector.memset(C9[:], 0.0)
    for k in range(3):
        for dx in (-1, 0, 1):
            j = k * 3 + (dx + 1)
            nc.vector.scalar_tensor_tensor(
                out=W9[:, j:j + 1], in0=rvk[:, k:k + 1], scalar=W4M1, in1=colok[dx][:],
                op0=OP.mult, op1=OP.mult)
        nc.vector.tensor_copy(C9[:, k * 3 + 1:k * 3 + 2], mvk[:, k:k + 1])
    nc.vector.tensor_scalar(out=C9[:], in0=C9[:], scalar1=float(W4M1 + 1.0), scalar2=None,
                            op0=OP.mult)
    nc.vector.tensor_tensor(out=W9[:], in0=W9[:], in1=C9[:], op=OP.subtract)

    # focal terms at the 9 patch pixels
    e9 = sm_tile([BS, 9], f32, "e9")
    nc.scalar.activation(e9[:], X[:], AF.Exp, scale=-1.0)
    L9 = sm_tile([BS, 9], f32, "L9")
    nc.scalar.activation(L9[:], e9[:], AF.Ln, bias=1.0)
    R9 = sm_tile([BS, 9], f32, "R9")
    nc.scalar.activation(R9[:], L9[:], AF.Exp, scale=-2.0)
    t9 = sm_tile([BS, 9], f32, "t9")   # softplus(x)*p^2 = -log(1-p)p^2
    nc.vector.tensor_add(t9[:], X[:], L9[:])
    nc.vector.tensor_tensor(out=t9[:], in0=t9[:], in1=R9[:], op=OP.mult)

    scr9 = sm_tile([BS, 9], f32, "scr9")
    # corr = sum_j W9_j * (log(1-p)p^2)_j = -sum_j W9_j * t9_j
    nc.vector.scalar_tensor_tensor(
        out=scr9[:], in0=W9[:], scalar=-1.0, in1=t9[:],
        op0=OP.mult, op1=OP.mult, accum_out=partials[0:BS, 1:2])

    # pos = centermask * ln(p)*(1-p)^2 = -sum_j cm9_j * L9_j * e9_j^2 * R9_j
    u9 = sm_tile([BS, 9], f32, "u9")
    nc.vector.tensor_tensor(out=u9[:], in0=e9[:], in1=e9[:], op=OP.mult)
    nc.vector.tensor_tensor(out=u9[:], in0=u9[:], in1=R9[:], op=OP.mult)
    nc.vector.tensor_tensor(out=u9[:], in0=u9[:], in1=L9[:], op=OP.mult)
    cm9 = sm_tile([BS, 9], f32, "cm9")
    nc.vector.memset(cm9[:], 0.0)
    for k in range(3):
        nc.vector.tensor_copy(cm9[:, k * 3 + 1:k * 3 + 2], mvk[:, k:k + 1])
    nc.vector.scalar_tensor_tensor(
        out=scr9[:], in0=u9[:], scalar=-1.0, in1=cm9[:],
        op0=OP.mult, op1=OP.mult, accum_out=partials[0:BS, 2:3])

    # reg predictions: Rp[:, c-1] = sum_k mk * <slab[k, c, :], oh[0]>
    ohm = sm_tile([BS, 3 * W], f32, "ohm")
    for k in range(3):
        nc.vector.tensor_scalar(out=ohm[:, k * W:(k + 1) * W], in0=oh[0][:],
                                scalar1=mk[k][:, 0:1], scalar2=None, op0=OP.mult)
    Rp = sm_tile([BS, 6], f32, "Rp")
    pr3 = sm_tile([BS, 3 * W], f32, "pr3")
    for c in range(1, C):
        csl = slab[:].rearrange("p (k cx) -> p k cx", cx=RPB)[:, :, c * W:(c + 1) * W]
        nc.vector.tensor_tensor(out=pr3[:].rearrange("p (k x) -> p k x", x=W),
                                in0=csl, in1=ohm[:].rearrange("p (k x) -> p k x", x=W),
                                op=OP.mult)
        nc.vector.reduce_sum(out=Rp[:, c - 1:c], in_=pr3[:], axis=AX.X)

    # reg targets
    T = sm_tile([BS, 6], f32, "T")
    nc.vector.tensor_tensor(out=T[:, 0:2], in0=gtt[:, 1:3], in1=fl_f[:], op=OP.subtract)
    nc.scalar.activation(T[:, 2:3], gtt[:, 3:4], AF.Ln)
    nc.scalar.activation(T[:, 3:4], gtt[:, 4:5], AF.Ln)
    v = sm_tile([BS, 1], f32, "v")
    nc.vector.tensor_scalar(out=v[:], in0=gtt[:, 5:6], scalar1=float(-np.pi),
                            scalar2=None, op0=OP.add)
    v2 = sm_tile([BS, 1], f32, "v2")
    nc.vector.tensor_tensor(out=v2[:], in0=v[:], in1=v[:], op=OP.mult)

    def horner(coefs, dst_col, extra_mul=None):
        acc_t = sm_tile([BS, 1], f32, "hacc")
        nc.vector.memset(acc_t[:], float(coefs[-1]))
        for cf in coefs[-2::-1]:
            nc.vector.tensor_scalar(out=acc_t[:], in0=acc_t[:], scalar1=v2[:, 0:1],
                                    scalar2=float(cf), op0=OP.mult, op1=OP.add)
        if extra_mul is not None:
            nc.vector.tensor_tensor(out=acc_t[:], in0=acc_t[:], in1=extra_mul[:], op=OP.mult)
        nc.vector.tensor_scalar(out=dst_col, in0=acc_t[:], scalar1=-1.0,
                                scalar2=None, op0=OP.mult)

    horner(SIN_C, T[:, 4:5], extra_mul=v)     # sin(yaw) = -v*P(v^2)
    horner(COS_C, T[:, 5:6])                  # cos(yaw) = -Q(v^2)

    d6 = sm_tile([BS, 6], f32, "d6")
    nc.vector.tensor_tensor(out=d6[:], in0=Rp[:], in1=T[:], op=OP.subtract)
    nc.vector.tensor_scalar(out=d6[:], in0=d6[:], scalar1=vf[:, 0:1], scalar2=None, op0=OP.mult)
    nc.vector.tensor_reduce(out=partials[0:BS, 3:4], in_=d6[:], axis=AX.X,
                            op=OP.add, apply_absolute_value=True)
    nc.vector.tensor_copy(partials[0:BS, 4:5], vf[:])

    nc.sync.dma_start(out[:], partials[:])


_CACHE = {}


def _get_program():
    if "nc" not in _CACHE:
        nc = bacc.Bacc("TRN2", target_bir_lowering=False, debug=False,
                       num_devices=NCORES)
        preds = nc.dram_tensor("preds", [ROWS, RPB], f32, kind="ExternalInput").ap()
        gt = nc.dram_tensor("gt", [BS, 6], f32, kind="ExternalInput").ap()
        out = nc.dram_tensor("partials", [128, 8], f32, kind="ExternalOutput").ap()
        with tile.TileContext(nc) as tc:
            with ExitStack() as ctx:
                _body(ctx, tc, preds, gt, out)
        nc.compile()
        _CACHE["nc"] = nc
    return _CACHE["nc"]


def _combine(partials_list):
    s = np.zeros(8, np.float64)
    for p in partials_list:
        s += p.astype(np.float64).sum(axis=0)
    sum_mr, corr, pos, l1, npos = s[0] + s[5], s[1], s[2], s[3], s[4]
    neg = -sum_mr + corr
    if npos > 0:
        loss_hm = -(pos + neg) / max(npos, 1.0)
    else:
        loss_hm = -neg
    loss = loss_hm + 2.0 * (l1 / (npos + 1e-4))
    return np.asarray(loss, dtype=np.float32)


def _shard_inputs(preds, gt_boxes):
    """Per-core in_maps; preds shipped as the (BS*H, C*W) view of (b,y,c,x)."""
    preds_t = np.ascontiguousarray(preds.transpose(0, 2, 1, 3))  # (B,H,C,W)
    in_maps = []
    for i in range(NCORES):
        in_maps.append({
            "preds": preds_t[i * BS:(i + 1) * BS].reshape(ROWS, RPB),
            "gt": gt_boxes[i * BS:(i + 1) * BS],
        })
    return in_maps


def _get_executor():
    """Cached jitted shard_map executor (avoids per-call XLA recompiles)."""
    if "exec" in _CACHE:
        return _CACHE["exec"]
    import jax
    from jax.sharding import Mesh, PartitionSpec
    from jax.experimental.shard_map import shard_map
    from concourse import bass2jax

    nc = _get_program()
    bass2jax.install_neuronx_cc_hook()
    partition_name = nc.partition_id_tensor.name if nc.partition_id_tensor else None
    in_names, out_names, out_avals = [], [], []
    for alloc in nc.m.functions[0].allocations:
        if not isinstance(alloc, mybir.MemoryLocationSet):
            continue
        name = alloc.memorylocations[0].name
        if alloc.kind == "ExternalInput":
            if name != partition_name:
                in_names.append(name)
        elif alloc.kind == "ExternalOutput":
            out_names.append(name)
            out_avals.append(jax.core.ShapedArray(tuple(alloc.tensor_shape),
                                                  mybir.dt.np(alloc.dtype)))
    all_names = in_names + out_names + ([partition_name] if partition_name else [])

    def _body(*args):
        operands = list(args)
        if partition_name is not None:
            operands.append(bass2jax.partition_id_tensor())
        return tuple(bass2jax._bass_exec_p.bind(
            *operands, out_avals=tuple(out_avals), in_names=tuple(all_names),
            out_names=tuple(out_names), lowering_input_output_aliases=(),
            sim_require_finite=True, sim_require_nnan=True, nc=nc))

    devices = jax.devices()[:NCORES]
    mesh = Mesh(np.asarray(devices), ("core",))
    nin = len(in_names) + len(out_names)
    sharded = jax.jit(shard_map(
        _body, mesh=mesh, in_specs=(PartitionSpec("core"),) * nin,
        out_specs=(PartitionSpec("core"),) * len(out_names), check_rep=False))
    _CACHE["exec"] = (sharded, in_names, out_names, out_avals)
    return _CACHE["exec"]


def kernel(preds, gt_boxes):
    preds = np.ascontiguousarray(preds, dtype=np.float32)
    gt_boxes = np.ascontiguousarray(gt_boxes, dtype=np.float32)
    in_maps = _shard_inputs(preds, gt_boxes)
    if "exec" not in _CACHE and "first_done" not in _CACHE:
        # first call: run through the canonical bass_utils path
        from concourse.bass_utils import run_bass_kernel_spmd
        nc = _get_program()
        res = run_bass_kernel_spmd(nc, in_maps, list(range(NCORES)))
        _CACHE["first_done"] = True
        return _combine([r["partials"] for r in res.results])
    sharded, in_names, out_names, out_avals = _get_executor()
    concat_in = [np.concatenate([m[n] for m in in_maps], 0) for n in in_names]
    concat_zeros = [np.zeros((NCORES * a.shape[0], *a.shape[1:]), a.dtype)
                    for a in out_avals]
    outs = sharded(*concat_in, *concat_zeros)
    P = np.asarray(outs[0]).reshape(NCORES, *out_avals[0].shape)
    return _combine([P[c] for c in range(NCORES)])



# revision 2
# speedup vs baseline: 1.5847x; 1.5847x over previous
"""Trainium2 Bass kernel for CenterHead loss (data-parallel over batch, 8 cores).

Math notes
----------
reference loss = focal(sigmoid(preds[:,0]), target_hm) + 2 * L1(pred_reg, target_reg)

The target heatmap is 0 everywhere except a 3x3 patch per batch (center 1.0,
ring 0.8), and target_reg/mask are nonzero only at the center pixel. So:
  * neg-loss base: treat EVERY pixel of channel 0 as a t=0 negative:
      sum log(1-p) * p^2   over all pixels
    computed as -sum softplus(x) * p^2 with
      e = exp(-x); L = ln(1+e) (=softplus(-x)); p^2 = exp(-2L); softplus(x) = x+L
    (single ACT table set natural_log_exp_and_others; the hw_specs activation
    table cache is patched so Exp/Ln both resolve to it -- no table thrash)
  * corrections for the <=9 patch pixels per batch:
      ring pixel (t=0.8, in range):  weight changes 1 -> 0.2^4
      center (t=1.0): remove its neg term, add pos term ln(p)*(1-p)^2
  * reg L1 needs preds[b,1:7,cy,cx] plus targets from gt_boxes
    (floor/ln/sin-cos-poly computed on device).

Layout/perf:
  * channel 0 ships from host as packed bf16 "hm" [H, BS*W] (partition = y),
    so each streaming tile is a contiguous 4KB-per-partition DMA. The big-pass
    matmul reduction sum (x+L)*p^2 runs as two bf16 PSUM chains
    diag(x.T@R) + diag(L.T@R) (bf16 matmul is 4x fp32 rate; PSUM stays f32).
  * the full transposed f32 preds [BS*H, C*W] stays for the per-batch slab
    gather: rows start..start+2 (start = clip(cy-1,0,H-3)) of image (B,H,C,W)
    are one contiguous 3*C*W slab with the ch0 patch AND all six reg rows.
  * gt_boxes DMA + slab gather issue on the gpsimd queue before the big loop;
    the per-batch serial DVE chain is emitted mid-loop so it hides under the
    big-pass ACT work.

Per-core output "partials" [128, 8] f32 columns:
  0: diag(x.T@R) partial      5: diag(L.T@R) partial   (0+5 = softplus*p^2 sum)
  1: per-batch neg-loss correction     2: per-batch pos term
  3: per-batch reg L1                  4: per-batch valid flag
Host sums across partitions+cores and applies the final divisions.
"""
from contextlib import ExitStack

import numpy as np

import concourse.bass as bass
import concourse.bacc as bacc
import concourse.tile as tile
import concourse.mybir as mybir

f32 = mybir.dt.float32
bf16 = mybir.dt.bfloat16
i32 = mybir.dt.int32
AF = mybir.ActivationFunctionType
OP = mybir.AluOpType
AX = mybir.AxisListType

B, C, H, W = 512, 7, 128, 128
NCORES = 8
BS = B // NCORES            # 64 batches per core
RPB = C * W                 # 896 elems per (b,y) row in transposed layout
ROWS = BS * H               # 8192 rows of the [BS*H, C*W] view
NT = 4                      # streaming tiles
TB = BS // NT               # 16 batches per tile
FD = TB * H * W // 128      # 2048 free elems per partition per tile

W4M1 = float((1.0 - 0.8) ** 4 - 1.0)   # ring weight delta: (1-t)^4 - 1

# sin/cos via polynomial in u=v^2, v = yaw - pi in [-pi,pi]:
#   sin(yaw) = -v*P(u), cos(yaw) = -Q(u)
def _trig_coefs():
    import numpy.polynomial.chebyshev as cheb
    vg = np.linspace(-np.pi, np.pi, 20001)
    sin_c = np.polynomial.Polynomial(cheb.cheb2poly(cheb.chebfit(vg**2, np.sinc(vg / np.pi), 6))).coef
    cos_c = np.polynomial.Polynomial(cheb.cheb2poly(cheb.chebfit(vg**2, np.cos(vg), 7))).coef
    return [float(c) for c in sin_c], [float(c) for c in cos_c]

SIN_C, COS_C = _trig_coefs()


def _body(ctx: ExitStack, tc, hm, preds, gt, out):
    nc = tc.nc
    xp = ctx.enter_context(tc.tile_pool(name="xp", bufs=3))
    big = ctx.enter_context(tc.tile_pool(name="big", bufs=2))
    sm = ctx.enter_context(tc.tile_pool(name="sm", bufs=1))

    def _mk(pool):
        def f(shape, dtype, tag):
            return pool.tile(shape, dtype, tag=tag, name=tag)
        return f
    sm_tile, xp_tile, big_tile = _mk(sm), _mk(xp), _mk(big)

    partials = sm_tile([128, 8], f32, "partials")
    nc.vector.memset(partials[:], 0.0)

    # ---------------- per-batch prep (no slab dependency) ----------------
    gtt = sm_tile([BS, 6], f32, "gtt")
    nc.gpsimd.dma_start(gtt[:], gt[:])
    cxf, cyf = gtt[:, 1:2], gtt[:, 2:3]

    # floor of (cx, cy) together: round via f32->i32 copy, fix up if rf > src
    fl_i = sm_tile([BS, 2], i32, "fl_i")
    nc.vector.tensor_copy(fl_i[:], gtt[:, 1:3])
    fl_f = sm_tile([BS, 2], f32, "fl_f")
    nc.vector.tensor_copy(fl_f[:], fl_i[:])
    fl_fx = sm_tile([BS, 2], f32, "fl_fx")
    nc.vector.tensor_tensor(out=fl_fx[:], in0=fl_f[:], in1=gtt[:, 1:3], op=OP.is_gt)
    nc.vector.tensor_tensor(out=fl_f[:], in0=fl_f[:], in1=fl_fx[:], op=OP.subtract)
    nc.vector.tensor_copy(fl_i[:], fl_f[:])
    cx_f, cy_f = fl_f[:, 0:1], fl_f[:, 1:2]
    cy_i = fl_i[:, 1:2]

    # valid = 0 <= cx < W and 0 <= cy < H (W == H == 128 so one bound tile)
    vboth = sm_tile([BS, 2], f32, "vboth")
    vtmp = sm_tile([BS, 2], f32, "vtmp")
    nc.vector.tensor_scalar(out=vboth[:], in0=gtt[:, 1:3], scalar1=0.0, scalar2=None, op0=OP.is_ge)
    nc.vector.tensor_scalar(out=vtmp[:], in0=gtt[:, 1:3], scalar1=float(W), scalar2=None, op0=OP.is_lt)
    nc.vector.tensor_tensor(out=vboth[:], in0=vboth[:], in1=vtmp[:], op=OP.mult)
    vf = sm_tile([BS, 1], f32, "vf")
    nc.vector.tensor_tensor(out=vf[:], in0=vboth[:, 0:1], in1=vboth[:, 1:2], op=OP.mult)

    # slab start row: start = clip(cy-1, 0, H-3); gather row index = b*H + start
    st_i = sm_tile([BS, 1], i32, "st_i")
    nc.vector.tensor_scalar(out=st_i[:], in0=cy_i, scalar1=-1, scalar2=0,
                            op0=OP.add, op1=OP.max)
    nc.vector.tensor_scalar(out=st_i[:], in0=st_i[:], scalar1=H - 3, scalar2=None, op0=OP.min)
    st_f = sm_tile([BS, 1], f32, "st_f")
    nc.vector.tensor_copy(st_f[:], st_i[:])
    biota = sm_tile([BS, 1], i32, "biota")
    nc.gpsimd.iota(biota[:], pattern=[[0, 1]], base=0, channel_multiplier=H)
    gidx = sm_tile([BS, 1], i32, "gidx")
    nc.vector.tensor_tensor(out=gidx[:], in0=st_i[:], in1=biota[:], op=OP.add)

    # one slab gather: 3 view-rows (3*C*W elems) per batch
    slab = sm_tile([BS, 3 * RPB], f32, "slab")
    nc.gpsimd.indirect_dma_start(
        out=slab[:], out_offset=None, in_=preds[:],
        in_offset=bass.IndirectOffsetOnAxis(ap=gidx[:, 0:1], axis=0))

    def slab_ch(k, c):  # (BS, W) AP of slot k, channel c
        return slab[:, k * RPB + c * W: k * RPB + (c + 1) * W]

    # slot masks vs cy: mk = [y_k == cy], rowmask_k = [|y_k - cy| <= 1]
    mk, rowm = [], []
    for k in range(3):
        m = sm_tile([BS, 1], f32, f"mk{k}")
        nc.vector.tensor_scalar(out=m[:], in0=st_f[:], scalar1=float(k), scalar2=cy_f,
                                op0=OP.add, op1=OP.is_equal)
        mk.append(m)
        r1 = sm_tile([BS, 1], f32, f"rma{k}")
        nc.vector.tensor_scalar(out=r1[:], in0=st_f[:], scalar1=float(k + 1), scalar2=cy_f,
                                op0=OP.add, op1=OP.is_ge)
        r2 = sm_tile([BS, 1], f32, f"rmb{k}")
        nc.vector.tensor_scalar(out=r2[:], in0=st_f[:], scalar1=float(k - 1), scalar2=cy_f,
                                op0=OP.add, op1=OP.is_le)
        nc.vector.tensor_tensor(out=r1[:], in0=r1[:], in1=r2[:], op=OP.mult)
        rowm.append(r1)

    # col-ok masks and x-onehots per dx (onehot [x - dx == cx] needs no clip)
    iota_x = sm_tile([BS, W], i32, "iota_x")
    nc.gpsimd.iota(iota_x[:], pattern=[[1, W]], base=0, channel_multiplier=0)
    iota_xf = sm_tile([BS, W], f32, "iota_xf")
    nc.vector.tensor_copy(iota_xf[:], iota_x[:])
    oh, colok = {}, {}
    for dx in (-1, 0, 1):
        o = sm_tile([BS, W], f32, f"oh{dx}")
        nc.vector.tensor_scalar(out=o[:], in0=iota_xf[:], scalar1=float(-dx), scalar2=cx_f,
                                op0=OP.add, op1=OP.is_equal)
        oh[dx] = o
        ck1 = sm_tile([BS, 1], f32, f"cka{dx}")
        nc.vector.tensor_scalar(out=ck1[:], in0=cx_f, scalar1=float(dx), scalar2=0.0,
                                op0=OP.add, op1=OP.is_ge)
        ck2 = sm_tile([BS, 1], f32, f"ckb{dx}")
        nc.vector.tensor_scalar(out=ck2[:], in0=cx_f, scalar1=float(dx), scalar2=float(W - 1),
                                op0=OP.add, op1=OP.is_le)
        nc.vector.tensor_tensor(out=ck1[:], in0=ck1[:], in1=ck2[:], op=OP.mult)
        colok[dx] = ck1

    # weights: W9 = w4m1*basemask - (w4m1+1)*centermask
    #   basemask_j = rowmask_k * colok_dx * valid; centermask_j = mk * [dx==0] * valid
    W9 = sm_tile([BS, 9], f32, "W9")
    C9 = sm_tile([BS, 9], f32, "C9")
    rvk = sm_tile([BS, 3], f32, "rvk")
    mvk = sm_tile([BS, 3], f32, "mvk")
    for k in range(3):
        nc.vector.tensor_tensor(out=rvk[:, k:k + 1], in0=rowm[k][:], in1=vf[:], op=OP.mult)
        nc.vector.tensor_tensor(out=mvk[:, k:k + 1], in0=mk[k][:], in1=vf[:], op=OP.mult)
    nc.vector.memset(C9[:], 0.0)
    for k in range(3):
        for dx in (-1, 0, 1):
            j = k * 3 + (dx + 1)
            nc.vector.scalar_tensor_tensor(
                out=W9[:, j:j + 1], in0=rvk[:, k:k + 1], scalar=W4M1, in1=colok[dx][:],
                op0=OP.mult, op1=OP.mult)
        nc.vector.tensor_copy(C9[:, k * 3 + 1:k * 3 + 2], mvk[:, k:k + 1])
    nc.vector.tensor_scalar(out=C9[:], in0=C9[:], scalar1=float(W4M1 + 1.0), scalar2=None,
                            op0=OP.mult)
    nc.vector.tensor_tensor(out=W9[:], in0=W9[:], in1=C9[:], op=OP.subtract)

    # ---------------- big streaming pass over channel 0 ----------------
    # sum softplus(x)*p^2 = sum (x+L)*R accumulated on the PE as
    # diag(sum_chunks x_c.T @ R_c) + diag(sum_chunks L_c.T @ R_c), all bf16
    psum = ctx.enter_context(tc.tile_pool(name="psum", bufs=1, space="PSUM"))
    psA = psum.tile([128, 128], f32, tag="psA", name="psA")
    psB = psum.tile([128, 128], f32, tag="psB", name="psB")
    ident = sm_tile([128, 128], f32, "ident")
    from concourse.masks import make_identity
    make_identity(nc, ident[:])
    NCH = FD // 128

    def emit_tile(t):
        x = xp_tile([128, FD], bf16, "x")
        nc.sync.dma_start(x[:], hm[:, t * FD:(t + 1) * FD])
        e = big_tile([128, FD], f32, "e")
        nc.scalar.activation(e[:], x[:], AF.Exp, scale=-1.0)
        L = big_tile([128, FD], bf16, "L")
        nc.scalar.activation(L[:], e[:], AF.Ln, bias=1.0)
        R = big_tile([128, FD], bf16, "R")
        nc.scalar.activation(R[:], L[:], AF.Exp, scale=-2.0)
        for cchunk in range(NCH):
            cs = slice(cchunk * 128, (cchunk + 1) * 128)
            first = (t == 0 and cchunk == 0)
            last = (t == NT - 1 and cchunk == NCH - 1)
            nc.tensor.matmul(psA[:], x[:, cs], R[:, cs], start=first, stop=last)
            nc.tensor.matmul(psB[:], L[:, cs], R[:, cs], start=first, stop=last)

    def emit_slab_section():
        # extract the 9 patch logits X[:, j], j = k*3 + (dx+1)
        X = sm_tile([BS, 9], f32, "X")
        scr = sm_tile([BS, W], f32, "scr")
        for k in range(3):
            for dx in (-1, 0, 1):
                j = k * 3 + (dx + 1)
                nc.vector.scalar_tensor_tensor(
                    out=scr[:], in0=slab_ch(k, 0), scalar=1.0, in1=oh[dx][:],
                    op0=OP.mult, op1=OP.mult, accum_out=X[:, j:j + 1])

        # focal terms at the 9 patch pixels
        e9 = sm_tile([BS, 9], f32, "e9")
        nc.scalar.activation(e9[:], X[:], AF.Exp, scale=-1.0)
        L9 = sm_tile([BS, 9], f32, "L9")
        nc.scalar.activation(L9[:], e9[:], AF.Ln, bias=1.0)
        R9 = sm_tile([BS, 9], f32, "R9")
        nc.scalar.activation(R9[:], L9[:], AF.Exp, scale=-2.0)
        t9 = sm_tile([BS, 9], f32, "t9")   # softplus(x)*p^2 = -log(1-p)p^2
        nc.vector.tensor_add(t9[:], X[:], L9[:])
        nc.vector.tensor_tensor(out=t9[:], in0=t9[:], in1=R9[:], op=OP.mult)

        scr9 = sm_tile([BS, 9], f32, "scr9")
        # corr = sum_j W9_j * (log(1-p)p^2)_j = -sum_j W9_j * t9_j
        nc.vector.scalar_tensor_tensor(
            out=scr9[:], in0=W9[:], scalar=-1.0, in1=t9[:],
            op0=OP.mult, op1=OP.mult, accum_out=partials[0:BS, 1:2])

        # pos = centermask * ln(p)*(1-p)^2 = -sum_j cm9_j * L9_j * e9_j^2 * R9_j
        u9 = sm_tile([BS, 9], f32, "u9")
        nc.vector.tensor_tensor(out=u9[:], in0=e9[:], in1=e9[:], op=OP.mult)
        nc.vector.tensor_tensor(out=u9[:], in0=u9[:], in1=R9[:], op=OP.mult)
        nc.vector.tensor_tensor(out=u9[:], in0=u9[:], in1=L9[:], op=OP.mult)
        cm9 = sm_tile([BS, 9], f32, "cm9")
        nc.vector.memset(cm9[:], 0.0)
        for k in range(3):
            nc.vector.tensor_copy(cm9[:, k * 3 + 1:k * 3 + 2], mvk[:, k:k + 1])
        nc.vector.scalar_tensor_tensor(
            out=scr9[:], in0=u9[:], scalar=-1.0, in1=cm9[:],
            op0=OP.mult, op1=OP.mult, accum_out=partials[0:BS, 2:3])

        # reg predictions: Rp[:, c-1] = sum_k mk * <slab[k, c, :], oh[0]>
        ohm = sm_tile([BS, 3 * W], f32, "ohm")
        for k in range(3):
            nc.vector.tensor_scalar(out=ohm[:, k * W:(k + 1) * W], in0=oh[0][:],
                                    scalar1=mk[k][:, 0:1], scalar2=None, op0=OP.mult)
        Rp = sm_tile([BS, 6], f32, "Rp")
        pr3 = sm_tile([BS, 3 * W], f32, "pr3")
        for c in range(1, C):
            csl = slab[:].rearrange("p (k cx) -> p k cx", cx=RPB)[:, :, c * W:(c + 1) * W]
            nc.vector.tensor_tensor(out=pr3[:].rearrange("p (k x) -> p k x", x=W),
                                    in0=csl, in1=ohm[:].rearrange("p (k x) -> p k x", x=W),
                                    op=OP.mult)
            nc.vector.reduce_sum(out=Rp[:, c - 1:c], in_=pr3[:], axis=AX.X)

        # reg targets
        T = sm_tile([BS, 6], f32, "T")
        nc.vector.tensor_tensor(out=T[:, 0:2], in0=gtt[:, 1:3], in1=fl_f[:], op=OP.subtract)
        nc.scalar.activation(T[:, 2:3], gtt[:, 3:4], AF.Ln)
        nc.scalar.activation(T[:, 3:4], gtt[:, 4:5], AF.Ln)
        v = sm_tile([BS, 1], f32, "v")
        nc.vector.tensor_scalar(out=v[:], in0=gtt[:, 5:6], scalar1=float(-np.pi),
                                scalar2=None, op0=OP.add)
        v2 = sm_tile([BS, 1], f32, "v2")
        nc.vector.tensor_tensor(out=v2[:], in0=v[:], in1=v[:], op=OP.mult)

        def horner(coefs, dst_col, extra_mul=None):
            acc_t = sm_tile([BS, 1], f32, "hacc")
            nc.vector.memset(acc_t[:], float(coefs[-1]))
            for cf in coefs[-2::-1]:
                nc.vector.tensor_scalar(out=acc_t[:], in0=acc_t[:], scalar1=v2[:, 0:1],
                                        scalar2=float(cf), op0=OP.mult, op1=OP.add)
            if extra_mul is not None:
                nc.vector.tensor_tensor(out=acc_t[:], in0=acc_t[:], in1=extra_mul[:], op=OP.mult)
            nc.vector.tensor_scalar(out=dst_col, in0=acc_t[:], scalar1=-1.0,
                                    scalar2=None, op0=OP.mult)

        horner(SIN_C, T[:, 4:5], extra_mul=v)     # sin(yaw) = -v*P(v^2)
        horner(COS_C, T[:, 5:6])                  # cos(yaw) = -Q(v^2)

        d6 = sm_tile([BS, 6], f32, "d6")
        nc.vector.tensor_tensor(out=d6[:], in0=Rp[:], in1=T[:], op=OP.subtract)
        nc.vector.tensor_scalar(out=d6[:], in0=d6[:], scalar1=vf[:, 0:1], scalar2=None, op0=OP.mult)
        nc.vector.tensor_reduce(out=partials[0:BS, 3:4], in_=d6[:], axis=AX.X,
                                op=OP.add, apply_absolute_value=True)
        nc.vector.tensor_copy(partials[0:BS, 4:5], vf[:])

    for t in range(NT):
        emit_tile(t)
        if t == 1:
            emit_slab_section()

    scrd = sm_tile([128, 128], f32, "scrd")
    nc.vector.scalar_tensor_tensor(
        out=scrd[:], in0=psA[:], scalar=1.0, in1=ident[:],
        op0=OP.mult, op1=OP.mult, accum_out=partials[:, 0:1])
    nc.vector.scalar_tensor_tensor(
        out=scrd[:], in0=psB[:], scalar=1.0, in1=ident[:],
        op0=OP.mult, op1=OP.mult, accum_out=partials[:, 5:6])

    nc.sync.dma_start(out[:], partials[:])


_CACHE = {}


def _patch_act_tables(arch):
    """Make Exp and Ln both resolve to natural_log_exp_and_others so the
    compiler emits a single ACT table load instead of thrashing between
    exp_and_others and natural_log (2.7us per switch)."""
    from concourse.hw_specs import get_activation_tables
    tabs = get_activation_tables(arch)
    for name in ("exp_and_others", "natural_log", "exp_and_friends"):
        if name in tabs and "natural_log_exp_and_others" in tabs:
            tabs[name].discard(AF.Exp)
            tabs[name].discard(AF.Ln)


def _get_program():
    if "nc" not in _CACHE:
        nc = bacc.Bacc("TRN2", target_bir_lowering=False, debug=False,
                       num_devices=NCORES)
        _patch_act_tables(nc.m.arch)
        hm = nc.dram_tensor("hm", [H, BS * W], bf16, kind="ExternalInput").ap()
        preds = nc.dram_tensor("preds", [ROWS, RPB], f32, kind="ExternalInput").ap()
        gt = nc.dram_tensor("gt", [BS, 6], f32, kind="ExternalInput").ap()
        out = nc.dram_tensor("partials", [128, 8], f32, kind="ExternalOutput").ap()
        with tile.TileContext(nc) as tc:
            with ExitStack() as ctx:
                _body(ctx, tc, hm, preds, gt, out)
        nc.compile()
        _CACHE["nc"] = nc
    return _CACHE["nc"]


def _combine(partials_list):
    s = np.zeros(8, np.float64)
    for p in partials_list:
        s += p.astype(np.float64).sum(axis=0)
    sum_mr, corr, pos, l1, npos = s[0] + s[5], s[1], s[2], s[3], s[4]
    neg = -sum_mr + corr
    if npos > 0:
        loss_hm = -(pos + neg) / max(npos, 1.0)
    else:
        loss_hm = -neg
    loss = loss_hm + 2.0 * (l1 / (npos + 1e-4))
    return np.asarray(loss, dtype=np.float32)


def _shard_inputs(preds, gt_boxes):
    """Per-core in_maps; preds shipped as the (BS*H, C*W) view of (b,y,c,x)
    for the slab gather, plus channel 0 as packed bf16 [H, BS*W]."""
    bft = mybir.dt.np(bf16)
    preds_t = np.ascontiguousarray(preds.transpose(0, 2, 1, 3))  # (B,H,C,W)
    in_maps = []
    for i in range(NCORES):
        pc = preds_t[i * BS:(i + 1) * BS]                        # (BS,H,C,W)
        hm_c = np.ascontiguousarray(
            pc[:, :, 0, :].transpose(1, 0, 2)).reshape(H, BS * W).astype(bft)
        in_maps.append({
            "hm": hm_c,
            "preds": pc.reshape(ROWS, RPB),
            "gt": gt_boxes[i * BS:(i + 1) * BS],
        })
    return in_maps


def _get_executor():
    """Cached jitted shard_map executor (avoids per-call XLA recompiles)."""
    if "exec" in _CACHE:
        return _CACHE["exec"]
    import jax
    from jax.sharding import Mesh, PartitionSpec
    from jax.experimental.shard_map import shard_map
    from concourse import bass2jax

    nc = _get_program()
    bass2jax.install_neuronx_cc_hook()
    partition_name = nc.partition_id_tensor.name if nc.partition_id_tensor else None
    in_names, out_names, out_avals = [], [], []
    for alloc in nc.m.functions[0].allocations:
        if not isinstance(alloc, mybir.MemoryLocationSet):
            continue
        name = alloc.memorylocations[0].name
        if alloc.kind == "ExternalInput":
            if name != partition_name:
                in_names.append(name)
        elif alloc.kind == "ExternalOutput":
            out_names.append(name)
            out_avals.append(jax.core.ShapedArray(tuple(alloc.tensor_shape),
                                                  mybir.dt.np(alloc.dtype)))
    all_names = in_names + out_names + ([partition_name] if partition_name else [])

    def _body(*args):
        operands = list(args)
        if partition_name is not None:
            operands.append(bass2jax.partition_id_tensor())
        return tuple(bass2jax._bass_exec_p.bind(
            *operands, out_avals=tuple(out_avals), in_names=tuple(all_names),
            out_names=tuple(out_names), lowering_input_output_aliases=(),
            sim_require_finite=True, sim_require_nnan=True, nc=nc))

    devices = jax.devices()[:NCORES]
    mesh = Mesh(np.asarray(devices), ("core",))
    nin = len(in_names) + len(out_names)
    sharded = jax.jit(shard_map(
        _body, mesh=mesh, in_specs=(PartitionSpec("core"),) * nin,
        out_specs=(PartitionSpec("core"),) * len(out_names), check_rep=False))
    _CACHE["exec"] = (sharded, in_names, out_names, out_avals)
    return _CACHE["exec"]


def kernel(preds, gt_boxes):
    preds = np.ascontiguousarray(preds, dtype=np.float32)
    gt_boxes = np.ascontiguousarray(gt_boxes, dtype=np.float32)
    in_maps = _shard_inputs(preds, gt_boxes)
    if "exec" not in _CACHE and "first_done" not in _CACHE:
        # first call: run through the canonical bass_utils path
        from concourse.bass_utils import run_bass_kernel_spmd
        nc = _get_program()
        res = run_bass_kernel_spmd(nc, in_maps, list(range(NCORES)))
        _CACHE["first_done"] = True
        return _combine([r["partials"] for r in res.results])
    sharded, in_names, out_names, out_avals = _get_executor()
    concat_in = [np.concatenate([m[n] for m in in_maps], 0) for n in in_names]
    concat_zeros = [np.zeros((NCORES * a.shape[0], *a.shape[1:]), a.dtype)
                    for a in out_avals]
    outs = sharded(*concat_in, *concat_zeros)
    P = np.asarray(outs[0]).reshape(NCORES, *out_avals[0].shape)
    return _combine([P[c] for c in range(NCORES)])


# revision 6
# speedup vs baseline: 1.7299x; 1.0916x over previous
"""Trainium2 Bass kernel for CenterHead loss (data-parallel over batch, 8 cores).

Math notes
----------
reference loss = focal(sigmoid(preds[:,0]), target_hm) + 2 * L1(pred_reg, target_reg)

The target heatmap is 0 everywhere except a 3x3 patch per batch (center 1.0,
ring 0.8), and target_reg/mask are nonzero only at the center pixel. So:
  * neg-loss base: treat EVERY pixel of channel 0 as a t=0 negative:
      sum log(1-p) * p^2  over all pixels,  p = sigmoid(x)
    computed in TWO ACT passes with one table switch:
      phase A: p = Sigmoid(x)            (sigmoid_and_others set)
      phase B: z = Ln(1 - p)             (natural_log_exp_and_others set)
    q = p*p on DVE (bf16); sum z*q accumulated on the PE as a single bf16
    PSUM chain diag(z.T @ q).  (The baseline needed 3 passes: Exp/Ln/Exp.)
  * corrections for the <=9 patch pixels per batch:
      ring pixel (t=0.8, in range):  weight changes 1 -> 0.2^4
      center (t=1.0): remove its neg term, add pos term ln(p)*(1-p)^2
  * reg L1 needs preds[b,1:7,cy,cx] plus targets from gt_boxes
    (floor/ln/sin-cos-poly computed on device).

Layout/perf:
  * channel 0 ships from host as packed bf16 "hm" [H, BS*W] (partition = y),
    so each streaming tile is a contiguous >=3KB-per-partition DMA.
  * uneven streaming tiles [12,24,24,4] batches: small first tile lets ACT
    start early; tiny last tile shrinks the PE/extract tail after the final
    ACT pass (total ACT time is invariant to the split).
  * the full transposed f32 preds [BS*H, C*W] stays for the per-batch slab
    gather: rows start..start+2 (start = clip(cy-1,0,H-3)) of image (B,H,C,W)
    are one contiguous 3*C*W slab with the ch0 patch AND all six reg rows.
  * gt_boxes DMA + slab gather issue on the gpsimd queue before the big loop;
    the reg-channel extraction runs on the Pool engine (gpsimd) and the rest
    of the per-batch serial chain hides under big-pass ACT work.

Per-core output "partials" [128, 8] f32 columns:
  0: sum softplus(x)*p^2 partial (= -diag(z.T@q))
  1: per-batch neg-loss correction     2: per-batch pos term
  3: per-batch reg L1                  4: per-batch valid flag
Host sums across partitions+cores and applies the final divisions.
"""
from contextlib import ExitStack

import numpy as np

import concourse.bass as bass
import concourse.bacc as bacc
import concourse.tile as tile
import concourse.mybir as mybir

f32 = mybir.dt.float32
bf16 = mybir.dt.bfloat16
i32 = mybir.dt.int32
AF = mybir.ActivationFunctionType
OP = mybir.AluOpType
AX = mybir.AxisListType

B, C, H, W = 512, 7, 128, 128
NCORES = 8
BS = B // NCORES            # 64 batches per core
RPB = C * W                 # 896 elems per (b,y) row in transposed layout
ROWS = BS * H               # 8192 rows of the [BS*H, C*W] view
TBS = [12, 24, 24, 4]       # uneven streaming tiles (batches per tile)
NT = len(TBS)
OFFS = [sum(TBS[:i]) * W for i in range(NT + 1)]   # hm column offsets

W4M1 = float((1.0 - 0.8) ** 4 - 1.0)   # ring weight delta: (1-t)^4 - 1

# sin/cos via polynomial in u=v^2, v = yaw - pi in [-pi,pi]:
#   sin(yaw) = -v*P(u), cos(yaw) = -Q(u)
def _trig_coefs():
    import numpy.polynomial.chebyshev as cheb
    vg = np.linspace(-np.pi, np.pi, 20001)
    sin_c = np.polynomial.Polynomial(cheb.cheb2poly(cheb.chebfit(vg**2, np.sinc(vg / np.pi), 6))).coef
    cos_c = np.polynomial.Polynomial(cheb.cheb2poly(cheb.chebfit(vg**2, np.cos(vg), 7))).coef
    return [float(c) for c in sin_c], [float(c) for c in cos_c]

SIN_C, COS_C = _trig_coefs()


def _body(ctx: ExitStack, tc, hm, preds, gt, out):
    nc = tc.nc
    sm = ctx.enter_context(tc.tile_pool(name="sm", bufs=1))

    def sm_tile(shape, dtype, tag):
        return sm.tile(shape, dtype, tag=tag, name=tag)

    partials = sm_tile([128, 8], f32, "partials")
    nc.vector.memset(partials[:], 0.0)

    # ---------------- per-batch prep (no slab dependency) ----------------
    gtt = sm_tile([BS, 6], f32, "gtt")
    nc.gpsimd.dma_start(gtt[:], gt[:])
    cxf, cyf = gtt[:, 1:2], gtt[:, 2:3]

    # floor of (cx, cy) together: round via f32->i32 copy, fix up if rf > src
    fl_i = sm_tile([BS, 2], i32, "fl_i")
    nc.vector.tensor_copy(fl_i[:], gtt[:, 1:3])
    fl_f = sm_tile([BS, 2], f32, "fl_f")
    nc.vector.tensor_copy(fl_f[:], fl_i[:])
    fl_fx = sm_tile([BS, 2], f32, "fl_fx")
    nc.vector.tensor_tensor(out=fl_fx[:], in0=fl_f[:], in1=gtt[:, 1:3], op=OP.is_gt)
    nc.vector.tensor_tensor(out=fl_f[:], in0=fl_f[:], in1=fl_fx[:], op=OP.subtract)
    nc.vector.tensor_copy(fl_i[:], fl_f[:])
    cx_f, cy_f = fl_f[:, 0:1], fl_f[:, 1:2]
    cy_i = fl_i[:, 1:2]

    # valid = 0 <= cx < W and 0 <= cy < H (W == H == 128 so one bound tile)
    vboth = sm_tile([BS, 2], f32, "vboth")
    vtmp = sm_tile([BS, 2], f32, "vtmp")
    nc.vector.tensor_scalar(out=vboth[:], in0=gtt[:, 1:3], scalar1=0.0, scalar2=None, op0=OP.is_ge)
    nc.vector.tensor_scalar(out=vtmp[:], in0=gtt[:, 1:3], scalar1=float(W), scalar2=None, op0=OP.is_lt)
    nc.vector.tensor_tensor(out=vboth[:], in0=vboth[:], in1=vtmp[:], op=OP.mult)
    vf = sm_tile([BS, 1], f32, "vf")
    nc.vector.tensor_tensor(out=vf[:], in0=vboth[:, 0:1], in1=vboth[:, 1:2], op=OP.mult)

    # slab start row: start = clip(cy-1, 0, H-3); gather row index = b*H + start
    st_i = sm_tile([BS, 1], i32, "st_i")
    nc.vector.tensor_scalar(out=st_i[:], in0=cy_i, scalar1=-1, scalar2=0,
                            op0=OP.add, op1=OP.max)
    nc.vector.tensor_scalar(out=st_i[:], in0=st_i[:], scalar1=H - 3, scalar2=None, op0=OP.min)
    st_f = sm_tile([BS, 1], f32, "st_f")
    nc.vector.tensor_copy(st_f[:], st_i[:])
    biota = sm_tile([BS, 1], i32, "biota")
    nc.gpsimd.iota(biota[:], pattern=[[0, 1]], base=0, channel_multiplier=H)
    gidx = sm_tile([BS, 1], i32, "gidx")
    nc.vector.tensor_tensor(out=gidx[:], in0=st_i[:], in1=biota[:], op=OP.add)

    # one slab gather: 3 view-rows (3*C*W elems) per batch
    slab = sm_tile([BS, 3 * RPB], f32, "slab")
    nc.gpsimd.indirect_dma_start(
        out=slab[:], out_offset=None, in_=preds[:],
        in_offset=bass.IndirectOffsetOnAxis(ap=gidx[:, 0:1], axis=0))

    def slab_ch(k, c):  # (BS, W) AP of slot k, channel c
        return slab[:, k * RPB + c * W: k * RPB + (c + 1) * W]

    # slot masks vs cy: mk = [y_k == cy], rowmask_k = [|y_k - cy| <= 1]
    mk, rowm = [], []
    for k in range(3):
        m = sm_tile([BS, 1], f32, f"mk{k}")
        nc.vector.tensor_scalar(out=m[:], in0=st_f[:], scalar1=float(k), scalar2=cy_f,
                                op0=OP.add, op1=OP.is_equal)
        mk.append(m)
        r1 = sm_tile([BS, 1], f32, f"rma{k}")
        nc.vector.tensor_scalar(out=r1[:], in0=st_f[:], scalar1=float(k + 1), scalar2=cy_f,
                                op0=OP.add, op1=OP.is_ge)
        r2 = sm_tile([BS, 1], f32, f"rmb{k}")
        nc.vector.tensor_scalar(out=r2[:], in0=st_f[:], scalar1=float(k - 1), scalar2=cy_f,
                                op0=OP.add, op1=OP.is_le)
        nc.vector.tensor_tensor(out=r1[:], in0=r1[:], in1=r2[:], op=OP.mult)
        rowm.append(r1)

    # col-ok masks and x-onehots per dx (onehot [x - dx == cx] needs no clip)
    iota_x = sm_tile([BS, W], i32, "iota_x")
    nc.gpsimd.iota(iota_x[:], pattern=[[1, W]], base=0, channel_multiplier=0)
    iota_xf = sm_tile([BS, W], f32, "iota_xf")
    nc.vector.tensor_copy(iota_xf[:], iota_x[:])
    oh, colok = {}, {}
    for dx in (-1, 0, 1):
        o = sm_tile([BS, W], f32, f"oh{dx}")
        nc.vector.tensor_scalar(out=o[:], in0=iota_xf[:], scalar1=float(-dx), scalar2=cx_f,
                                op0=OP.add, op1=OP.is_equal)
        oh[dx] = o
        ck1 = sm_tile([BS, 1], f32, f"cka{dx}")
        nc.vector.tensor_scalar(out=ck1[:], in0=cx_f, scalar1=float(dx), scalar2=0.0,
                                op0=OP.add, op1=OP.is_ge)
        ck2 = sm_tile([BS, 1], f32, f"ckb{dx}")
        nc.vector.tensor_scalar(out=ck2[:], in0=cx_f, scalar1=float(dx), scalar2=float(W - 1),
                                op0=OP.add, op1=OP.is_le)
        nc.vector.tensor_tensor(out=ck1[:], in0=ck1[:], in1=ck2[:], op=OP.mult)
        colok[dx] = ck1

    # weights: W9 = w4m1*basemask - (w4m1+1)*centermask
    #   basemask_j = rowmask_k * colok_dx * valid; centermask_j = mk * [dx==0] * valid
    W9 = sm_tile([BS, 9], f32, "W9")
    C9 = sm_tile([BS, 9], f32, "C9")
    rvk = sm_tile([BS, 3], f32, "rvk")
    mvk = sm_tile([BS, 3], f32, "mvk")
    for k in range(3):
        nc.vector.tensor_tensor(out=rvk[:, k:k + 1], in0=rowm[k][:], in1=vf[:], op=OP.mult)
        nc.vector.tensor_tensor(out=mvk[:, k:k + 1], in0=mk[k][:], in1=vf[:], op=OP.mult)
    nc.vector.memset(C9[:], 0.0)
    for k in range(3):
        for dx in (-1, 0, 1):
            j = k * 3 + (dx + 1)
            nc.vector.scalar_tensor_tensor(
                out=W9[:, j:j + 1], in0=rvk[:, k:k + 1], scalar=W4M1, in1=colok[dx][:],
                op0=OP.mult, op1=OP.mult)
        nc.vector.tensor_copy(C9[:, k * 3 + 1:k * 3 + 2], mvk[:, k:k + 1])
    nc.vector.tensor_scalar(out=C9[:], in0=C9[:], scalar1=float(W4M1 + 1.0), scalar2=None,
                            op0=OP.mult)
    nc.vector.tensor_tensor(out=W9[:], in0=W9[:], in1=C9[:], op=OP.subtract)

    cm9 = sm_tile([BS, 9], f32, "cm9")
    nc.vector.memset(cm9[:], 0.0)
    for k in range(3):
        nc.vector.tensor_copy(cm9[:, k * 3 + 1:k * 3 + 2], mvk[:, k:k + 1])

    # sin/cos targets + dx/dy (phase-independent DVE work)
    T = sm_tile([BS, 6], f32, "T")
    nc.vector.tensor_tensor(out=T[:, 0:2], in0=gtt[:, 1:3], in1=fl_f[:], op=OP.subtract)
    v = sm_tile([BS, 1], f32, "v")
    nc.vector.tensor_scalar(out=v[:], in0=gtt[:, 5:6], scalar1=float(-np.pi),
                            scalar2=None, op0=OP.add)
    v2 = sm_tile([BS, 1], f32, "v2")
    nc.vector.tensor_tensor(out=v2[:], in0=v[:], in1=v[:], op=OP.mult)

    def horner(coefs, dst_col, extra_mul=None):
        acc_t = sm_tile([BS, 1], f32, "hacc")
        nc.vector.memset(acc_t[:], float(coefs[-1]))
        for cf in coefs[-2::-1]:
            nc.vector.tensor_scalar(out=acc_t[:], in0=acc_t[:], scalar1=v2[:, 0:1],
                                    scalar2=float(cf), op0=OP.mult, op1=OP.add)
        if extra_mul is not None:
            nc.vector.tensor_tensor(out=acc_t[:], in0=acc_t[:], in1=extra_mul[:], op=OP.mult)
        nc.vector.tensor_scalar(out=dst_col, in0=acc_t[:], scalar1=-1.0,
                                scalar2=None, op0=OP.mult)

    horner(SIN_C, T[:, 4:5], extra_mul=v)     # sin(yaw) = -v*P(v^2)
    horner(COS_C, T[:, 5:6])                  # cos(yaw) = -Q(v^2)

    # reg predictions on the Pool engine: Rp[:, c-1] = sum_k mk * <slab[k,c,:], oh[0]>
    ohm = sm_tile([BS, 3 * W], f32, "ohm")
    for k in range(3):
        nc.gpsimd.tensor_scalar(out=ohm[:, k * W:(k + 1) * W], in0=oh[0][:],
                                scalar1=mk[k][:, 0:1], scalar2=None, op0=OP.mult)
    Rp = sm_tile([BS, 6], f32, "Rp")
    pr3 = sm_tile([BS, 3 * W], f32, "pr3")

    def emit_reg_extract():
        for c in range(1, C):
            csl = slab[:].rearrange("p (k cx) -> p k cx", cx=RPB)[:, :, c * W:(c + 1) * W]
            nc.vector.scalar_tensor_tensor(
                out=pr3[:].rearrange("p (k x) -> p k x", x=W), in0=csl, scalar=1.0,
                in1=ohm[:].rearrange("p (k x) -> p k x", x=W),
                op0=OP.mult, op1=OP.mult, accum_out=Rp[:, c - 1:c])

    # ---------------- big pass phase A: p = sigmoid(x), q = p^2 ----------------
    psum = ctx.enter_context(tc.tile_pool(name="psum", bufs=1, space="PSUM"))
    psA = psum.tile([128, 128], f32, tag="psA", name="psA")
    ident = sm_tile([128, 128], f32, "ident")
    from concourse.masks import make_identity
    make_identity(nc, ident[:])

    xs = [sm_tile([128, TBS[t] * W], bf16, f"x{t}") for t in range(NT)]
    ps = [sm_tile([128, TBS[t] * W], bf16, f"p{t}") for t in range(NT)]
    qs = [sm_tile([128, TBS[t] * W], bf16, f"q{t}") for t in range(NT)]
    zs = [sm_tile([128, TBS[t] * W], bf16, f"z{t}") for t in range(NT)]

    X = sm_tile([BS, 9], f32, "X")
    scr = sm_tile([BS, W], f32, "scr")

    for t in range(NT):
        nc.sync.dma_start(xs[t][:], hm[:, OFFS[t]:OFFS[t + 1]])
        nc.scalar.activation(ps[t][:], xs[t][:], AF.Sigmoid)
        nc.vector.tensor_tensor(out=qs[t][:], in0=ps[t][:], in1=ps[t][:], op=OP.mult)
        if t == 1:
            # extract the 9 patch logits X[:, j], j = k*3 + (dx+1)
            for k in range(3):
                for dx in (-1, 0, 1):
                    j = k * 3 + (dx + 1)
                    nc.vector.scalar_tensor_tensor(
                        out=scr[:], in0=slab_ch(k, 0), scalar=1.0, in1=oh[dx][:],
                        op0=OP.mult, op1=OP.mult, accum_out=X[:, j:j + 1])
            emit_reg_extract()

    # ---------------- phase B: z = ln(1-p), PE chain, patch terms ----------------
    # patch terms use exp/ln (same set 6 as z) so phase A stays pure sigmoid:
    #   e9 = exp(-X); L9 = ln(1+e9); R9 = exp(-2*L9) = p^2
    for t in range(NT):
        nc.scalar.activation(zs[t][:], ps[t][:], AF.Ln, scale=-1.0, bias=1.0)
        for cchunk in range(TBS[t]):
            cs = slice(cchunk * 128, (cchunk + 1) * 128)
            first = (t == 0 and cchunk == 0)
            last = (t == NT - 1 and cchunk == TBS[t] - 1)
            nc.tensor.matmul(psA[:], zs[t][:, cs], qs[t][:, cs], start=first, stop=last)
        if t == 1:
            e9 = sm_tile([BS, 9], f32, "e9")
            nc.scalar.activation(e9[:], X[:], AF.Exp, scale=-1.0)
            L9 = sm_tile([BS, 9], f32, "L9")
            nc.scalar.activation(L9[:], e9[:], AF.Ln, bias=1.0)
            R9 = sm_tile([BS, 9], f32, "R9")
            nc.scalar.activation(R9[:], L9[:], AF.Exp, scale=-2.0)
            nc.scalar.activation(T[:, 2:4], gtt[:, 3:5], AF.Ln)

            # corr col1 = -sum_j W9_j * softplus(X)*p^2
            t9 = sm_tile([BS, 9], f32, "t9")
            nc.vector.tensor_add(t9[:], X[:], L9[:])
            nc.vector.tensor_tensor(out=t9[:], in0=t9[:], in1=R9[:], op=OP.mult)
            scr9 = sm_tile([BS, 9], f32, "scr9")
            nc.vector.scalar_tensor_tensor(
                out=scr9[:], in0=W9[:], scalar=-1.0, in1=t9[:],
                op0=OP.mult, op1=OP.mult, accum_out=partials[0:BS, 1:2])

            # pos col2 = -sum_j cm9_j * L9*e9^2*R9  (= +sum cm9*ln(p)(1-p)^2)
            u9 = sm_tile([BS, 9], f32, "u9")
            nc.vector.tensor_tensor(out=u9[:], in0=e9[:], in1=e9[:], op=OP.mult)
            nc.vector.tensor_tensor(out=u9[:], in0=u9[:], in1=R9[:], op=OP.mult)
            nc.vector.tensor_tensor(out=u9[:], in0=u9[:], in1=L9[:], op=OP.mult)
            nc.vector.scalar_tensor_tensor(
                out=scr9[:], in0=u9[:], scalar=-1.0, in1=cm9[:],
                op0=OP.mult, op1=OP.mult, accum_out=partials[0:BS, 2:3])

            # reg L1 col3 + valid col4
            d6 = sm_tile([BS, 6], f32, "d6")
            nc.vector.tensor_tensor(out=d6[:], in0=Rp[:], in1=T[:], op=OP.subtract)
            nc.vector.tensor_scalar(out=d6[:], in0=d6[:], scalar1=vf[:, 0:1], scalar2=None, op0=OP.mult)
            nc.vector.tensor_reduce(out=partials[0:BS, 3:4], in_=d6[:], axis=AX.X,
                                    op=OP.add, apply_absolute_value=True)
            nc.vector.tensor_copy(partials[0:BS, 4:5], vf[:])

    # col0 = -diag(z.T@q) = +sum softplus(x)*p^2
    scrd = sm_tile([128, 128], f32, "scrd")
    nc.vector.scalar_tensor_tensor(
        out=scrd[:], in0=psA[:], scalar=-1.0, in1=ident[:],
        op0=OP.mult, op1=OP.mult, accum_out=partials[:, 0:1])

    nc.sync.dma_start(out[:], partials[:])


_CACHE = {}


def _patch_act_tables(arch):
    """Make Exp and Ln both resolve to natural_log_exp_and_others so the
    compiler never thrashes between exp_and_others and natural_log."""
    from concourse.hw_specs import get_activation_tables
    tabs = get_activation_tables(arch)
    for name in ("exp_and_others", "natural_log", "exp_and_friends"):
        if name in tabs and "natural_log_exp_and_others" in tabs:
            tabs[name].discard(AF.Exp)
            tabs[name].discard(AF.Ln)


def _get_program():
    if "nc" not in _CACHE:
        nc = bacc.Bacc("TRN2", target_bir_lowering=False, debug=False,
                       num_devices=NCORES)
        _patch_act_tables(nc.m.arch)
        hm = nc.dram_tensor("hm", [H, BS * W], bf16, kind="ExternalInput").ap()
        preds = nc.dram_tensor("preds", [ROWS, RPB], f32, kind="ExternalInput").ap()
        gt = nc.dram_tensor("gt", [BS, 6], f32, kind="ExternalInput").ap()
        out = nc.dram_tensor("partials", [128, 8], f32, kind="ExternalOutput").ap()
        with tile.TileContext(nc) as tc:
            with ExitStack() as ctx:
                _body(ctx, tc, hm, preds, gt, out)
        nc.compile()
        _CACHE["nc"] = nc
    return _CACHE["nc"]


def _combine(partials_list):
    s = np.zeros(8, np.float64)
    for p in partials_list:
        s += p.astype(np.float64).sum(axis=0)
    sum_mr, corr, pos, l1, npos = s[0] + s[5], s[1], s[2], s[3], s[4]
    neg = -sum_mr + corr
    if npos > 0:
        loss_hm = -(pos + neg) / max(npos, 1.0)
    else:
        loss_hm = -neg
    loss = loss_hm + 2.0 * (l1 / (npos + 1e-4))
    return np.asarray(loss, dtype=np.float32)


def _shard_inputs(preds, gt_boxes):
    """Per-core in_maps; preds shipped as the (BS*H, C*W) view of (b,y,c,x)
    for the slab gather, plus channel 0 as packed bf16 [H, BS*W]."""
    bft = mybir.dt.np(bf16)
    preds_t = np.ascontiguousarray(preds.transpose(0, 2, 1, 3))  # (B,H,C,W)
    in_maps = []
    for i in range(NCORES):
        pc = preds_t[i * BS:(i + 1) * BS]                        # (BS,H,C,W)
        hm_c = np.ascontiguousarray(
            pc[:, :, 0, :].transpose(1, 0, 2)).reshape(H, BS * W).astype(bft)
        in_maps.append({
            "hm": hm_c,
            "preds": pc.reshape(ROWS, RPB),
            "gt": gt_boxes[i * BS:(i + 1) * BS],
        })
    return in_maps


def _get_executor():
    """Cached jitted shard_map executor (avoids per-call XLA recompiles)."""
    if "exec" in _CACHE:
        return _CACHE["exec"]
    import jax
    from jax.sharding import Mesh, PartitionSpec
    from jax.experimental.shard_map import shard_map
    from concourse import bass2jax

    nc = _get_program()
    bass2jax.install_neuronx_cc_hook()
    partition_name = nc.partition_id_tensor.name if nc.partition_id_tensor else None
    in_names, out_names, out_avals = [], [], []
    for alloc in nc.m.functions[0].allocations:
        if not isinstance(alloc, mybir.MemoryLocationSet):
            continue
        name = alloc.memorylocations[0].name
        if alloc.kind == "ExternalInput":
            if name != partition_name:
                in_names.append(name)
        elif alloc.kind == "ExternalOutput":
            out_names.append(name)
            out_avals.append(jax.core.ShapedArray(tuple(alloc.tensor_shape),
                                                  mybir.dt.np(alloc.dtype)))
    all_names = in_names + out_names + ([partition_name] if partition_name else [])

    def _body(*args):
        operands = list(args)
        if partition_name is not None:
            operands.append(bass2jax.partition_id_tensor())
        return tuple(bass2jax._bass_exec_p.bind(
            *operands, out_avals=tuple(out_avals), in_names=tuple(all_names),
            out_names=tuple(out_names), lowering_input_output_aliases=(),
            sim_require_finite=True, sim_require_nnan=True, nc=nc))

    devices = jax.devices()[:NCORES]
    mesh = Mesh(np.asarray(devices), ("core",))
    nin = len(in_names) + len(out_names)
    sharded = jax.jit(shard_map(
        _body, mesh=mesh, in_specs=(PartitionSpec("core"),) * nin,
        out_specs=(PartitionSpec("core"),) * len(out_names), check_rep=False))
    _CACHE["exec"] = (sharded, in_names, out_names, out_avals)
    return _CACHE["exec"]


def kernel(preds, gt_boxes):
    preds = np.ascontiguousarray(preds, dtype=np.float32)
    gt_boxes = np.ascontiguousarray(gt_boxes, dtype=np.float32)
    in_maps = _shard_inputs(preds, gt_boxes)
    if "exec" not in _CACHE and "first_done" not in _CACHE:
        # first call: run through the canonical bass_utils path
        from concourse.bass_utils import run_bass_kernel_spmd
        nc = _get_program()
        res = run_bass_kernel_spmd(nc, in_maps, list(range(NCORES)))
        _CACHE["first_done"] = True
        return _combine([r["partials"] for r in res.results])
    sharded, in_names, out_names, out_avals = _get_executor()
    concat_in = [np.concatenate([m[n] for m in in_maps], 0) for n in in_names]
    concat_zeros = [np.zeros((NCORES * a.shape[0], *a.shape[1:]), a.dtype)
                    for a in out_avals]
    outs = sharded(*concat_in, *concat_zeros)
    P = np.asarray(outs[0]).reshape(NCORES, *out_avals[0].shape)
    return _combine([P[c] for c in range(NCORES)])
